# revision 10
# baseline (speedup 1.0000x reference)
"""RBF kernel layer (retrieval_knn): out = exp(-||x - p||^2) for x [131072, 64]
against 512 prototypes, distributed data-parallel over 8 NeuronCores.

Math: exp(-dist2) = exp(2*S) where S[n,m] = cross[n,m] - p_sq[m]/2 - x_sq[n]/2,
computed entirely in two bf16 hi/lo-split GEMMs accumulating in fp32 PSUM:
  mm1: [xh_t; nxsq_h; nxsq_l; 1; 1].T @ [ph; 1; 1; npsq_h; npsq_l]  (K=68)
  mm2: [xh_t; xl_t].T @ [pl; ph]                                    (K=128)
where x = xh + xl, p = ph + pl (bf16 splits; the dropped xl@pl term is
~2^-18), npsq* = bf16 split of -p_sq/2, nxsq* = bf16 split of -x_sq/2.

Perf structure (v4):
- 126-row tiles (128 of them) + two 128-row tiles: SBUF partitions
  126-127 carry almost no store bytes. SDMA engine 15 (serving
  partitions 92-95/124-127) intermittently runs ~25% slow from port
  contention outside our control; shrinking its share keeps it off the
  critical path whether or not the slowdown manifests.
- x loaded as 17 independent chunk tiles so matmuls start after the
  first chunk lands (~2 us after preamble) instead of the full 4 MB.
- Output DRAM layout is partition-major [128, 130, M] (host transposes
  back): per-partition store data is contiguous -> 16 KB descriptors.
  SCHUNK tiles per store DMA (~2 MB), staged in SBUF by the EXP
  activation itself.
- Stores issue from nc.scalar (ACT's HWDGE ring): EXP -> store is
  same-engine program order; loads on nc.sync never queue behind them.
- nxsq rows live at SBUF quadrant starts {0,32,64,96} (DVE copy sources
  must be 32-aligned), spreading the load over two DMA engines; A-slot
  ones rows are DMA'd once (DVE cannot write at partition 66).
"""

import numpy as np

# Problem constants (hardcoded per harness contract; kernel.py is self-contained)
N = 131072
D = 64
M = 512
GAMMA = 1.0
NCORES = 8
NSHARD = N // NCORES  # 16384
P = 128
R = 126  # main tile row count (keeps partitions 126-127 nearly idle)
NT_MAIN = 128  # main tiles: 128 * 126 = 16128 rows
NT_TAIL = 2  # tail tiles of 128 rows: 256 rows
NT = NT_MAIN + NT_TAIL
K1 = D + 4  # mm1 contraction: 64 xh rows + 2 xsq rows + 2 ones rows
LHS_SLOTS = 4  # manual rotation slots for A (ones rows initialized once)
XCHUNK = 8  # x tiles per input chunk DMA
OCHUNK = 4  # output tiles per ACTIVATE (PSUM 4-bank group)
SCHUNK = 8  # output tiles per store DMA (~2 MB)
STG_BUFS = 5  # staging buffers in flight

SIZES = [R] * NT_MAIN + [P] * NT_TAIL
ROWBASE = [R * t for t in range(NT_MAIN)] + [
    R * NT_MAIN + P * j for j in range(NT_TAIL)
]
NXCOLS = ((NT + 3) // 4) * P  # nxsq col span per quadrant row pair

_cache = {}


def _tile_chunk(t):
    """(chunk index, column offset within chunk) for tile t."""
    if t < NT_MAIN:
        return t // XCHUNK, (t % XCHUNK) * R
    return NT_MAIN // XCHUNK, (t - NT_MAIN) * P


def _build_bass(nshard=NSHARD):
    import concourse.mybir as mybir
    import concourse.tile as tile
    from concourse import bacc

    f32 = mybir.dt.float32
    bf16 = mybir.dt.bfloat16
    assert nshard == R * NT_MAIN + P * NT_TAIL

    nc = bacc.Bacc(None, target_bir_lowering=False)
    # pre-transposed on host: [p, n] = [xh|xl] feature p of point n
    xhl_d = nc.dram_tensor("xhl", [P, nshard], bf16, kind="ExternalInput")
    # nxsq rows (-x_sq/2 hi, lo) for tile t: DRAM rows 2*(t%4), +1,
    # cols (t//4)*P.., width SIZES[t]
    nxsq_d = nc.dram_tensor("nxsq", [8, NXCOLS], bf16, kind="ExternalInput")
    ones_d = nc.dram_tensor("ones2", [2, P], bf16, kind="ExternalInput")
    rhs1_d = nc.dram_tensor("rhs1", [K1, M], bf16, kind="ExternalInput")
    rhs2_d = nc.dram_tensor("rhs2", [2 * D, M], bf16, kind="ExternalInput")
    # partition-major: out_d[p, t, m] = out row ROWBASE[t]+p (host transposes)
    out_d = nc.dram_tensor("out", [P, NT, M], f32, kind="ExternalOutput")

    with tile.TileContext(nc) as tc:
        with (
            tc.tile_pool(name="singles", bufs=1) as singles,
            tc.tile_pool(name="stg", bufs=STG_BUFS) as stgp,
            tc.tile_pool(name="ps_o", bufs=2, space="PSUM") as ps_o,
        ):
            # A slots [68, 128]: rows 0..63 = xh_t, 64..65 = per-tile
            # [-x_sq/2 hi; lo], 66..67 = ones (constant, DMA'd once).
            a_slots = []
            for j in range(LHS_SLOTS):
                A_sb = singles.tile([K1, P], bf16, name=f"A{j}")
                nc.sync.dma_start(A_sb[D + 2 : K1, :], ones_d[:])
                a_slots.append(A_sb)

            # x chunks as INDEPENDENT tiles: compute on chunk c waits only
            # for chunk c's DMA.
            nxc = NT_MAIN // XCHUNK
            x_tiles = [
                singles.tile([P, XCHUNK * R], bf16, name=f"X{c}")
                for c in range(nxc)
            ]
            x_tiles.append(singles.tile([P, NT_TAIL * P], bf16, name="Xt"))

            def load_x_chunk(c):
                if c < nxc:
                    cs = slice(c * XCHUNK * R, (c + 1) * XCHUNK * R)
                else:
                    cs = slice(NT_MAIN * R, nshard)
                nc.sync.dma_start(x_tiles[c][:], xhl_d[:, cs])

            load_x_chunk(0)
            rhs1_sb = singles.tile([K1, M], bf16)
            nc.sync.dma_start(rhs1_sb[:], rhs1_d[:])
            rhs2_sb = singles.tile([2 * D, M], bf16)
            nc.sync.dma_start(rhs2_sb[:], rhs2_d[:])
            nxsq_sb = singles.tile([P, NXCOLS], bf16)
            for q in range(4):
                nc.sync.dma_start(
                    nxsq_sb[32 * q : 32 * q + 2, :],
                    nxsq_d[2 * q : 2 * q + 2, :],
                )
            for c in range(1, nxc + 1):
                load_x_chunk(c)

            store_groups = [
                list(range(s * SCHUNK, (s + 1) * SCHUNK))
                for s in range(NT_MAIN // SCHUNK)
            ] + [list(range(NT_MAIN, NT))]

            for tile_list in store_groups:
                stg = stgp.tile([P, SCHUNK, M], f32, tag="stg")
                for gi in range(0, len(tile_list), OCHUNK):
                    group = tile_list[gi : gi + OCHUNK]
                    ng = len(group)
                    rg = SIZES[group[0]]
                    psum = ps_o.tile([P, OCHUNK, M], f32, tag="psum")
                    for k, t in enumerate(group):
                        r = SIZES[t]
                        c, off = _tile_chunk(t)
                        T = x_tiles[c][:, off : off + r]
                        A = a_slots[t % LHS_SLOTS]
                        nc.vector.tensor_copy(A[0:D, 0:r], T[0:D, :])
                        a0 = 32 * (t % 4)
                        c0 = (t // 4) * P
                        nc.vector.tensor_copy(
                            A[D : D + 2, 0:r],
                            nxsq_sb[a0 : a0 + 2, c0 : c0 + r],
                        )
                        nc.tensor.matmul(
                            psum[0:r, k, :],
                            A[:, 0:r],
                            rhs1_sb[:],
                            start=True,
                            stop=False,
                        )
                        nc.tensor.matmul(
                            psum[0:r, k, :], T, rhs2_sb[:],
                            start=False, stop=True,
                        )
                    # out = exp(2*S) over the group's PSUM banks at once
                    nc.scalar.activation(
                        stg[0:rg, gi : gi + ng, :],
                        psum[0:rg, 0:ng, :],
                        mybir.ActivationFunctionType.Exp,
                        bias=0.0,
                        scale=2.0,
                    )
                t0 = tile_list[0]
                rg = SIZES[t0]
                nc.scalar.dma_start(
                    out_d[0:rg, t0 : t0 + len(tile_list), :],
                    stg[0:rg, 0 : len(tile_list), :],
                )

    nc.finalize()
    return nc


def _get_nc():
    if "nc" not in _cache:
        _cache["nc"] = _build_bass()
    return _cache["nc"]


def _prep_core_arrays(x, prototypes, nshard):
    """Build per-core host arrays (xhl row-major, nxsq, rhs1/rhs2)."""
    import ml_dtypes

    bf = ml_dtypes.bfloat16
    x = np.ascontiguousarray(np.asarray(x, dtype=np.float32))
    prototypes = np.ascontiguousarray(np.asarray(prototypes, dtype=np.float32))

    xh = x.astype(bf)
    xl = (x - xh.astype(np.float32)).astype(bf)
    # [128, N]: rows 0..63 = xh features, 64..127 = xl features
    xhl_t = np.ascontiguousarray(
        np.concatenate([xh, xl], axis=1).T
    )

    nxsq = (-0.5 * (x.astype(np.float64) ** 2).sum(axis=1)).astype(np.float32)
    nxh = nxsq.astype(bf)
    nxl = (nxsq - nxh.astype(np.float32)).astype(bf)

    pt = prototypes.T.astype(np.float32)  # [64, 512]
    ph = pt.astype(bf)
    pl = (pt - ph.astype(np.float32)).astype(bf)

    p_sq = (prototypes.astype(np.float64) ** 2).sum(axis=1)  # [512]
    t = (-0.5 * p_sq).astype(np.float32)
    th = t.astype(bf)
    tl = (t - th.astype(np.float32)).astype(bf)

    ones = np.ones((1, M), dtype=bf)
    # row order matches A: [xh_t rows; nxsq h/l rows; ones rows]
    rhs1 = np.ascontiguousarray(
        np.concatenate([ph, ones, ones, th[None, :], tl[None, :]], axis=0)
    )  # [68, 512] bf16
    rhs2 = np.ascontiguousarray(np.concatenate([pl, ph], axis=0))  # [128, 512]

    ones2 = np.ones((2, P), dtype=bf)
    ncores = x.shape[0] // nshard
    in_maps = []
    for s in range(ncores):
        sl = slice(s * nshard, (s + 1) * nshard)
        nxh_s, nxl_s = nxh[sl], nxl[sl]
        nxsq_spread = np.zeros((8, NXCOLS), dtype=bf)
        for t_ in range(NT):
            q, g = t_ % 4, t_ // 4
            r = SIZES[t_]
            b = ROWBASE[t_]
            nxsq_spread[2 * q, g * P : g * P + r] = nxh_s[b : b + r]
            nxsq_spread[2 * q + 1, g * P : g * P + r] = nxl_s[b : b + r]
        in_maps.append(
            {
                "xhl": np.ascontiguousarray(xhl_t[:, sl]),
                "nxsq": nxsq_spread,
                "ones2": ones2,
                "rhs1": rhs1,
                "rhs2": rhs2,
            }
        )
    return in_maps


def _prep_inputs(x, prototypes):
    return _prep_core_arrays(x, prototypes, NSHARD)


def _run(inputs, trace=False):
    from concourse.bass_utils import run_bass_kernel_spmd

    in_maps = _prep_inputs(inputs["x"], inputs["prototypes"])
    nc = _get_nc()
    res = run_bass_kernel_spmd(
        nc, in_maps, core_ids=list(range(NCORES)), trace=trace
    )
    # out is partition-major [P, NT, M]; shard row ROWBASE[t]+p = out[p, t]
    parts = []
    for r_ in res.results:
        o = r_["out"]  # [P, NT, M]
        main = o[:R, :NT_MAIN, :].transpose(1, 0, 2).reshape(R * NT_MAIN, M)
        tail = o[:, NT_MAIN:, :].transpose(1, 0, 2).reshape(P * NT_TAIL, M)
        parts.append(main)
        parts.append(tail)
    return np.ascontiguousarray(np.concatenate(parts, axis=0)), res


def kernel(**inputs) -> np.ndarray:
    out, _ = _run(inputs, trace=False)
    return out


# revision 13
# speedup vs baseline: 1.1172x; 1.1172x over previous
"""RBF kernel layer (retrieval_knn): out = exp(-||x - p||^2) for x [131072, 64]
against 512 prototypes, distributed data-parallel over 8 NeuronCores.

Math: exp(-dist2) = exp(2*S) where S[n,m] = cross[n,m] - p_sq[m]/2 - x_sq[n]/2,
computed entirely in two bf16 hi/lo-split GEMMs accumulating in fp32 PSUM:
  mm1: [xh_t; nxsq_h; nxsq_l; 1; 1].T @ [ph; 1; 1; npsq_h; npsq_l]  (K=68)
  mm2: [xh_t; xl_t].T @ [pl; ph]                                    (K=128)
where x = xh + xl, p = ph + pl (bf16 splits; the dropped xl@pl term is
~2^-18), npsq* = bf16 split of -p_sq/2, nxsq* = bf16 split of -x_sq/2.

Perf structure (v7):
- x loaded as 16 independent 256 KB chunk tiles, X0 issued before the
  small tables, so matmuls start right after the first chunk lands
  instead of after the full 4 MB load.
- nxsq ([4, nshard] on partitions 0-3, all served by one DMA engine) is
  loaded in 4 column chunks so tile 0's A-copy waits only for the first
  32 KB of it, not all 128 KB.
- Output DRAM layout is partition-major [128, nt, M] (host transposes
  back): per-partition store data is contiguous -> 16 KB descriptors.
  SCHUNK tiles per store DMA (2 MB), staged in SBUF by the EXP
  activation itself. All store/compute APs keep the full 128 partitions:
  the SDMA engine dealing is only balanced for 128-partition APs.
- Stores issue from nc.scalar (ACT's HWDGE ring): EXP -> store is
  same-engine program order; loads on nc.sync never queue behind them.
- Deep staging (STG_BUFS) decouples the producer from the slowest DMA
  engine so no SDMA engine idles waiting for queued work.
"""

import numpy as np

# Problem constants (hardcoded per harness contract; kernel.py is self-contained)
N = 131072
D = 64
M = 512
GAMMA = 1.0
NCORES = 8
NSHARD = N // NCORES  # 16384
P = 128
K1 = D + 4  # mm1 contraction: 64 xh rows + 2 xsq rows + 2 ones rows
LHS_SLOTS = 4  # manual rotation slots for A
XCHUNK = 8  # x tiles per input chunk DMA (256 KB)
NXQ = 4  # nxsq load chunks
OCHUNK = 4  # output tiles per ACTIVATE (PSUM 4-bank group)
SCHUNK = 8  # output tiles per store DMA (2 MB)
STG_BUFS = 5  # staging buffers in flight

_cache = {}


def _build_bass(nshard=NSHARD):
    import concourse.mybir as mybir
    import concourse.tile as tile
    from concourse import bacc

    f32 = mybir.dt.float32
    bf16 = mybir.dt.bfloat16
    nt = nshard // P
    assert nt % XCHUNK == 0 and nt % SCHUNK == 0 and SCHUNK % OCHUNK == 0

    nc = bacc.Bacc(None, target_bir_lowering=False)
    # pre-transposed on host: [p, i*P + j] = [xh|xl] feature p of point i*P+j
    xhl_d = nc.dram_tensor("xhl", [P, nshard], bf16, kind="ExternalInput")
    # nxsq rows (-x_sq/2 hi, lo) for tile i: DRAM rows 2*(i%4), +1,
    # cols (i//4)*P (loaded to SBUF quadrant starts {0,32,64,96}: DVE copy
    # sources must be 32-aligned; spreads the load over two DMA engines)
    nxsq_d = nc.dram_tensor("nxsq", [8, (nt // 4) * P], bf16, kind="ExternalInput")
    ones_d = nc.dram_tensor("ones2", [2, P], bf16, kind="ExternalInput")
    # rhs2 in cols 0..511 (128 rows), rhs1 in cols 512..1023 (rows 0..67):
    # one full-128-partition DMA keeps its descriptors balanced over all
    # 16 SDMA engines instead of piling 1 KB descriptors on engines 0-3
    rhs_d = nc.dram_tensor("rhs", [P, 2 * M], bf16, kind="ExternalInput")
    # partition-major: out_d[p, t, m] = out row t*P+p, col m (host transposes)
    out_d = nc.dram_tensor("out", [P, nt, M], f32, kind="ExternalOutput")

    with tile.TileContext(nc) as tc:
        with (
            tc.tile_pool(name="singles", bufs=1) as singles,
            tc.tile_pool(name="stg", bufs=STG_BUFS) as stgp,
            tc.tile_pool(name="ps_o", bufs=2, space="PSUM") as ps_o,
        ):
            # x chunks as INDEPENDENT tiles: compute on chunk c only waits
            # for chunk c's DMA. X0 goes first so compute starts ~2 us
            # after the preamble.
            x_tiles = [
                singles.tile([P, XCHUNK * P], bf16, name=f"X{c}")
                for c in range(nt // XCHUNK)
            ]

            def load_x_chunk(c):
                cs = slice(c * XCHUNK * P, (c + 1) * XCHUNK * P)
                nc.sync.dma_start(x_tiles[c][:], xhl_d[:, cs])

            # all x chunks stream back-to-back on the SP ring so every
            # SDMA engine is continuously fed from the start; the small
            # tables go on the ACT ring in parallel.
            for c in range(nt // XCHUNK):
                load_x_chunk(c)

            rhs_sb = singles.tile([P, 2 * M], bf16)
            nc.scalar.dma_start(rhs_sb[:], rhs_d[:])
            rhs1_sb = rhs_sb[0:K1, M : 2 * M]
            rhs2_sb = rhs_sb[:, 0:M]

            # A slots [68, 128]: rows 0..63 = xh_t, 64..65 = per-tile
            # [-x_sq/2 hi; lo], 66..67 = ones (constant, DMA'd once: DVE
            # cannot write at partition 66, DMA can).
            a_slots = []
            for j in range(LHS_SLOTS):
                A_sb = singles.tile([K1, P], bf16, name=f"A{j}")
                nc.scalar.dma_start(A_sb[D + 2 : K1, :], ones_d[:])
                a_slots.append(A_sb)

            # nxsq at SBUF quadrant starts {0,32,64,96}, 2 rows each
            nxsq_sb = singles.tile([P, (nt // 4) * P], bf16)
            for q in range(4):
                nc.scalar.dma_start(
                    nxsq_sb[32 * q : 32 * q + 2, :],
                    nxsq_d[2 * q : 2 * q + 2, :],
                )

            for i in range(nt):
                c, col = divmod(i, XCHUNK)
                k = i % OCHUNK
                j = i % SCHUNK
                if k == 0:
                    psum = ps_o.tile([P, OCHUNK, M], f32, tag="psum")
                if j == 0:
                    stg = stgp.tile([P, SCHUNK, M], f32, tag="stg")

                Xc = x_tiles[c]
                T = Xc[:, col * P : (col + 1) * P]
                A = a_slots[i % LHS_SLOTS]
                nc.vector.tensor_copy(A[0:D, :], T[0:D, :])
                a0 = 32 * (i % 4)
                c0 = (i // 4) * P
                nc.vector.tensor_copy(
                    A[D : D + 2, :], nxsq_sb[a0 : a0 + 2, c0 : c0 + P]
                )
                nc.tensor.matmul(
                    psum[:, k, :], A[:], rhs1_sb, start=True, stop=False
                )
                nc.tensor.matmul(
                    psum[:, k, :], T, rhs2_sb, start=False, stop=True
                )

                if k == OCHUNK - 1:
                    # out = exp(2*S) over all OCHUNK PSUM banks at once,
                    # written straight into the staging slot for the store
                    g = j // OCHUNK
                    nc.scalar.activation(
                        stg[:, g * OCHUNK : (g + 1) * OCHUNK, :],
                        psum[:],
                        mybir.ActivationFunctionType.Exp,
                        bias=0.0,
                        scale=2.0,
                    )
                    if j == SCHUNK - 1:
                        i0 = i - (SCHUNK - 1)
                        nc.scalar.dma_start(
                            out_d[:, i0 : i0 + SCHUNK, :], stg[:]
                        )

    nc.finalize()
    return nc


def _get_nc():
    if "nc" not in _cache:
        _cache["nc"] = _build_bass()
    return _cache["nc"]


def _prep_core_arrays(x, prototypes, nshard):
    """Build per-core host arrays (xhl row-major, nxsq, rhs1/rhs2)."""
    import ml_dtypes

    bf = ml_dtypes.bfloat16
    x = np.ascontiguousarray(np.asarray(x, dtype=np.float32))
    prototypes = np.ascontiguousarray(np.asarray(prototypes, dtype=np.float32))

    xh = x.astype(bf)
    xl = (x - xh.astype(np.float32)).astype(bf)
    # [128, N]: rows 0..63 = xh features, 64..127 = xl features
    xhl_t = np.ascontiguousarray(
        np.concatenate([xh, xl], axis=1).T
    )

    nxsq = (-0.5 * (x.astype(np.float64) ** 2).sum(axis=1)).astype(np.float32)
    nxh = nxsq.astype(bf)
    nxl = (nxsq - nxh.astype(np.float32)).astype(bf)

    pt = prototypes.T.astype(np.float32)  # [64, 512]
    ph = pt.astype(bf)
    pl = (pt - ph.astype(np.float32)).astype(bf)

    p_sq = (prototypes.astype(np.float64) ** 2).sum(axis=1)  # [512]
    t = (-0.5 * p_sq).astype(np.float32)
    th = t.astype(bf)
    tl = (t - th.astype(np.float32)).astype(bf)

    ones = np.ones((1, M), dtype=bf)
    # row order matches A: [xh_t rows; nxsq h/l rows; ones rows]
    rhs1 = np.ascontiguousarray(
        np.concatenate([ph, ones, ones, th[None, :], tl[None, :]], axis=0)
    )  # [68, 512] bf16
    rhs2 = np.ascontiguousarray(np.concatenate([pl, ph], axis=0))  # [128, 512]

    # merged rhs [128, 1024]: cols 0..511 = rhs2, cols 512..1023 rows
    # 0..67 = rhs1 (rest zero)
    rhs_all = np.zeros((P, 2 * M), dtype=bf)
    rhs_all[:, 0:M] = rhs2
    rhs_all[0:K1, M : 2 * M] = rhs1
    rhs_all = np.ascontiguousarray(rhs_all)
    ones2 = np.ones((2, P), dtype=bf)

    ncores = x.shape[0] // nshard
    nt = nshard // P
    in_maps = []
    for s in range(ncores):
        sl = slice(s * nshard, (s + 1) * nshard)
        nxsq_r = np.stack([nxh[sl], nxl[sl]], axis=0)  # [2, nshard]
        # spread layout [8, (nt//4)*P]: tile i's 2 rows at DRAM rows
        # 2*(i%4)..+1, cols (i//4)*P..
        t2 = nxsq_r.reshape(2, nt, P).transpose(1, 0, 2)  # [nt, 2, P]
        t2 = t2.reshape(nt // 4, 4, 2, P).transpose(1, 2, 0, 3)
        nxsq_spread = np.ascontiguousarray(t2.reshape(8, (nt // 4) * P))
        in_maps.append(
            {
                "xhl": np.ascontiguousarray(xhl_t[:, sl]),
                "nxsq": nxsq_spread,
                "ones2": ones2,
                "rhs": rhs_all,
            }
        )
    return in_maps


def _prep_inputs(x, prototypes):
    return _prep_core_arrays(x, prototypes, NSHARD)


def _run(inputs, trace=False):
    from concourse.bass_utils import run_bass_kernel_spmd

    in_maps = _prep_inputs(inputs["x"], inputs["prototypes"])
    nc = _get_nc()
    res = run_bass_kernel_spmd(
        nc, in_maps, core_ids=list(range(NCORES)), trace=trace
    )
    # out is partition-major [P, nt, M]; row t*P+p of the shard = out[p, t]
    out = np.concatenate(
        [
            r["out"].transpose(1, 0, 2).reshape(NSHARD, M)
            for r in res.results
        ],
        axis=0,
    )
    return np.ascontiguousarray(out), res


def kernel(**inputs) -> np.ndarray:
    out, _ = _run(inputs, trace=False)
    return out
